# revision 51
# baseline (speedup 1.0000x reference)
"""Trainium2 Bass kernel for nn_AnatomicalContrastiveLoss.

Distribution: V (voxel) dim sharded 8 ways; every core holds all B,C,F.
Per core:
  - weights = prod_c proba (sequential assoc, bit-exact with jnp.prod)
  - local top-48 per batch via iterated DVE max8/match_replace (2-level)
  - class_sum/count partials via PE matmuls over xbar-transposed bf16 embeddings
  - AllGather #1 (candidates) -> replicated global top-104 merge
  - index recovery via max_index against pristine weights; owner core gathers
    he/y rows by indirect DMA and computes the loss rows it owns
  - AllGather #2 (class_sum/count) -> EMA repr -> loss epilogue
Host combines per-core loss partials (disjoint support) and slices [:, :100].
"""

import numpy as np

import concourse.bass as bass
import concourse.tile as tile
from concourse import bacc, mybir
from concourse.bass import IndirectOffsetOnAxis
from concourse.masks import make_identity

B, C, V, F, K = 4, 4, 262144, 64, 100
NCORES = 8
VL = V // NCORES          # 32768
R1 = 3                    # L1 rounds: top-24 per 1024-voxel partition
R2 = 4                    # L2 rounds: top-32 per core
R3 = 13                   # L3 rounds: top-104 global
KE = 8 * R3               # 104 extracted globally
EMA_THETA = 0.9
TAU = 0.1
SENT = -1.0               # removal sentinel (< any weight; weights are in (0,1))
NOTFOUND_CLAMP = 2.0e7    # > B*F*VL, makes unowned gather indices OOB-skipped

f32 = mybir.dt.float32
bf16 = mybir.dt.bfloat16
i32 = mybir.dt.int32
u32 = mybir.dt.uint32
fp8 = mybir.dt.float8e4
Alu = mybir.AluOpType
Act = mybir.ActivationFunctionType
Axis = mybir.AxisListType


def build_graph(debug_taps=False):
    nc = bacc.Bacc("TRN2", target_bir_lowering=False, debug=False,
                   num_devices=NCORES)

    proba_d = nc.dram_tensor("proba", [B, C, VL], f32, kind="ExternalInput")
    y_d = nc.dram_tensor("y", [B, C, VL], i32, kind="ExternalInput")
    emb_d = nc.dram_tensor("embeddings", [B, F, VL], f32, kind="ExternalInput")
    avg_d = nc.dram_tensor("avg_repr", [1, C, F], f32, kind="ExternalInput")
    out_d = nc.dram_tensor("out", [B, KE], f32, kind="ExternalOutput")

    taps = {}

    def tap(name, shape, dtype=f32):
        if not debug_taps:
            return None
        t = nc.dram_tensor(name, shape, dtype, kind="ExternalOutput")
        taps[name] = t
        return t.ap()

    with tile.TileContext(nc) as tc:
        _build(tc, proba_d.ap(), y_d.ap(), emb_d.ap(), avg_d.ap(), out_d.ap(),
               tap)

    nc.compile()
    return nc


def _build(tc, proba_d, y_d, emb_d, avg_d, out_d, tap=lambda *a, **k: None):
    nc = tc.nc

    def emit_tap(name, src_ap, shape, dtype=f32):
        t = tap(name, shape, dtype)
        if t is not None:
            nc.scalar.dma_start(t, src_ap)

    import contextlib
    ctx = contextlib.ExitStack()
    with ctx:
        sb = ctx.enter_context(tc.tile_pool(name="sb", bufs=1))
        sb2 = ctx.enter_context(tc.tile_pool(name="sb2", bufs=2))
        sb3 = ctx.enter_context(tc.tile_pool(name="sb3", bufs=3))
        ps = ctx.enter_context(tc.tile_pool(name="ps", bufs=2, space="PSUM"))
        dram = ctx.enter_context(tc.tile_pool(name="dram", bufs=1,
                                              space="DRAM"))

        embT_dram = dram.tile([B * VL, F], fp8)
        s_dram = dram.tile([B * VL, 1], f32)

        # ============ input loads ============
        y_sb = sb.tile([128, 4096], i32)
        for c in range(C):
            nc.gpsimd.dma_start(y_sb[32 * c:32 * (c + 1), :], y_d[:, c, :])
        proba_sb = sb.tile([128, C, 1024], f32)
        for c in range(C):
            nc.gpsimd.dma_start(proba_sb[:, c, :], proba_d[:, c, :])

        # ============ constants ============
        ident = sb.tile([128, 128], f32)
        make_identity(nc, ident[:])
        ident_bf = sb.tile([128, 128], bf16)
        nc.vector.tensor_copy(ident_bf[:], ident[:])

        L4 = sb.tile([B, 128], f32)
        nc.gpsimd.memset(L4[:], 1.0)
        nc.gpsimd.affine_select(out=L4[:], in_=L4[:], compare_op=Alu.is_ge,
                                fill=0.0, base=0, channel_multiplier=-32,
                                pattern=[[1, 128]])
        nc.gpsimd.affine_select(out=L4[:], in_=L4[:], compare_op=Alu.is_ge,
                                fill=0.0, base=31, channel_multiplier=32,
                                pattern=[[-1, 128]])

        G2 = sb.tile([128, C], f32)
        nc.gpsimd.memset(G2[:], 1.0)
        nc.gpsimd.affine_select(out=G2[:], in_=G2[:], compare_op=Alu.is_ge,
                                fill=0.0, base=31, channel_multiplier=-1,
                                pattern=[[32, C]])
        nc.gpsimd.affine_select(out=G2[:], in_=G2[:], compare_op=Alu.is_ge,
                                fill=0.0, base=0, channel_multiplier=1,
                                pattern=[[-32, C]])

        S4 = sb.tile([128, 32], bf16)
        nc.gpsimd.memset(S4[:], 0.0)
        for c in range(C):
            nc.gpsimd.affine_select(out=S4[:], in_=S4[:],
                                    compare_op=Alu.not_equal, fill=1.0,
                                    base=-32 * c, channel_multiplier=1,
                                    pattern=[[-1, 32]])

        ones8 = sb.tile([NCORES, 1], f32)
        nc.gpsimd.memset(ones8[:], 1.0)
        ones104 = sb.tile([1, KE], f32)
        nc.gpsimd.memset(ones104[:], 1.0)

        p1024_i = sb.tile([128, 1], i32)
        nc.gpsimd.iota(p1024_i[:], pattern=[[1, 1]], base=0,
                       channel_multiplier=1024)
        p1024 = sb.tile([128, 1], f32)
        nc.vector.tensor_copy(p1024[:], p1024_i[:])
        bvals_i = sb.tile([B, 1], i32)
        nc.gpsimd.iota(bvals_i[:], pattern=[[1, 1]], base=0,
                       channel_multiplier=VL)
        bvals = sb.tile([B, 1], f32)
        nc.vector.tensor_copy(bvals[:], bvals_i[:])
        bsub_ps = ps.tile([128, 1], f32, tag="psa")
        nc.tensor.matmul(bsub_ps[:], lhsT=L4[:], rhs=bvals[:],
                         start=True, stop=True)
        bsub = sb.tile([128, 1], f32)
        nc.vector.tensor_copy(bsub[:], bsub_ps[:])
        # bcol[k, b] = b*VL (for batched index remap)
        bcol_i = sb.tile([KE, B], i32)
        nc.gpsimd.iota(bcol_i[:], pattern=[[1, B]], base=0,
                       channel_multiplier=0)
        bcol = sb.tile([KE, B], f32)
        nc.vector.tensor_copy(bcol[:], bcol_i[:])
        nc.vector.tensor_scalar_mul(bcol[:], bcol[:], float(VL))

        # ============ candidate path ============
        Worig = sb.tile([128, 1024], f32)
        nc.vector.tensor_tensor(Worig[:], proba_sb[:, 0, :],
                                proba_sb[:, 1, :], op=Alu.mult)
        nc.vector.tensor_tensor(Worig[:], Worig[:], proba_sb[:, 2, :],
                                op=Alu.mult)
        nc.vector.tensor_tensor(Worig[:], Worig[:], proba_sb[:, 3, :],
                                op=Alu.mult)
        emit_tap("d_W", Worig[:], [128, 1024])
        Wx = sb.tile([128, 1024], f32)
        nc.vector.tensor_copy(Wx[:], Worig[:])

        V1 = sb.tile([128, 8 * R1], f32)
        for r in range(R1):
            sl = V1[:, r * 8:(r + 1) * 8]
            nc.vector.max(out=sl, in_=Wx[:])
            nc.vector.match_replace(out=Wx[:], in_to_replace=sl,
                                    in_values=Wx[:], imm_value=SENT)

        cand = sb.tile([B, 32 * 8 * R1], f32)
        L2V = sb.tile([B, 8 * R2], f32)
        b1_in = dram.tile([B, 8 * R2], f32)
        b1_out = dram.tile([NCORES, B, 8 * R2], f32, addr_space="Shared")

        # ============ mask + maskT ============
        mask_bf = sb.tile([128, 4096], bf16)
        nc.scalar.activation(mask_bf[:], y_sb[:], Act.Sign)

        maskT = sb.tile([128, 4096], bf16)
        for g in range(4):
            mt_ps = ps.tile([128, 1024], bf16, tag="pset", bufs=4)
            for s8 in range(8):
                s = g * 8 + s8
                nc.tensor.transpose(mt_ps[:, s8 * 128:(s8 + 1) * 128],
                                    mask_bf[:, s * 128:(s + 1) * 128],
                                    ident_bf[:])
            nc.scalar.activation(maskT[:, g * 1024:(g + 1) * 1024], mt_ps[:],
                                 Act.Copy)
        emit_tap("d_maskT", maskT[:, 0:128], [128, 128], bf16)

        # sel-field s[b,v] -> s_dram
        msum = sb.tile([32, 4096], f32)
        for q in range(8):
            ms_ps = ps.tile([32, 512], f32, tag="psa")
            nc.tensor.matmul(ms_ps[:], lhsT=S4[:],
                             rhs=mask_bf[:, 512 * q:512 * (q + 1)],
                             start=True, stop=True)
            nc.vector.tensor_copy(msum[:, 512 * q:512 * (q + 1)], ms_ps[:])
        m0f = sb.tile([32, 4096], f32)
        nc.scalar.activation(m0f[:], mask_bf[0:32, :], Act.Copy)
        sval = sb.tile([32, 4096], f32)
        nc.vector.scalar_tensor_tensor(sval[:], in0=msum[:], scalar=0.0,
                                       in1=m0f[:], op0=Alu.is_equal,
                                       op1=Alu.max)
        cnt_part = sb.tile([128, 1], f32)
        nc.vector.reduce_sum(cnt_part[:], mask_bf[:], axis=Axis.X)
        cnt_ps = ps.tile([C, 1], f32, tag="psa")
        nc.tensor.matmul(cnt_ps[:], lhsT=G2[:], rhs=cnt_part[:],
                         start=True, stop=True)
        cnt_sb = sb.tile([C, 1], f32)
        nc.vector.tensor_copy(cnt_sb[:], cnt_ps[:])
        emit_tap("d_cnt", cnt_sb[:], [C, 1])

        # ============ embeddings stream machinery ============
        CH = 8192
        NCHUNK = B * VL // CH

        ebT_tiles = {}
        cs_psum = ps.tile([C, F], f32, tag="pscs", bufs=1)

        def stream_chunk(ci):
            b, h = ci // (VL // CH), ci % (VL // CH)
            ebf = sb3.tile([F, CH], bf16, tag="ebf", name=f"ebf{ci}")
            nc.gpsimd.dma_start(ebf[:], emb_d[b, :, h * CH:(h + 1) * CH])
            ebT = sb3.tile([128, CH // 128, F], bf16, tag="ebT", bufs=3,
                           name=f"ebT{ci}")
            for g in range(4):
                tp_ps = ps.tile([128, 1024], bf16, tag="pset", bufs=4,
                                name=f"tp{ci}_{g}")
                for t8 in range(16):
                    t = g * 16 + t8
                    nc.tensor.transpose(tp_ps[:, t8 * 64:(t8 + 1) * 64],
                                        ebf[:, t * 128:(t + 1) * 128],
                                        ident_bf[0:F, 0:F])
                dstv = ebT[:, g * 16:(g + 1) * 16, :]
                nc.scalar.activation(dstv, tp_ps[:], Act.Copy)
            ebT_tiles[ci] = ebT
            if ci >= 2:
                cj = ci - 2
                rj = (cj // (VL // CH)) * VL + (cj % (VL // CH)) * CH
                nc.gpsimd.dma_start(embT_dram[rj:rj + CH, :],
                                    ebT_tiles.pop(cj)[:])
            for t in range(CH // 128):
                v0 = h * CH + t * 128
                ch8, s = v0 // 4096, (v0 % 4096) // 128
                col0 = s * 128 + b * 8 + ch8
                lhsT = maskT[:, col0:col0 + 32 * (C - 1) + 1:32]
                nc.tensor.matmul(cs_psum[:], lhsT=lhsT, rhs=ebT[:, t, :],
                                 start=(ci == 0 and t == 0),
                                 stop=(ci == NCHUNK - 1 and t == CH // 128 - 1),
                                 skip_group_check=True)

        for ci in range(0, 3):
            stream_chunk(ci)

        for ci in range(3, 5):
            stream_chunk(ci)

        nc.sync.dma_start(cand[:], V1[:])
        for r in range(R2):
            sl = L2V[:, r * 8:(r + 1) * 8]
            nc.vector.max(out=sl, in_=cand[:])
            nc.vector.match_replace(out=cand[:], in_to_replace=sl,
                                    in_values=cand[:], imm_value=SENT)
        emit_tap("d_L2V", L2V[:], [B, 8 * R2])
        nc.sync.dma_start(b1_in[:], L2V[:])

        for ci in range(5, 9):
            stream_chunk(ci)

        nc.gpsimd.collective_compute(
            "AllGather", Alu.bypass,
            replica_groups=[list(range(NCORES))],
            ins=[b1_in[:].opt()], outs=[b1_out[:].opt()])

        for ci in range(9, NCHUNK):
            stream_chunk(ci)
        for cj in sorted(ebT_tiles):
            rj = (cj // (VL // CH)) * VL + (cj % (VL // CH)) * CH
            nc.gpsimd.dma_start(embT_dram[rj:rj + CH, :],
                                ebT_tiles.pop(cj)[:])


        nc.sync.dma_start(s_dram[:].rearrange("(p v) a -> p (v a)", p=32),
                          sval[:])
        # ============ AllGather #2 ============
        pack2 = sb.tile([C, F + 1], f32)
        nc.vector.tensor_copy(pack2[:, 0:F], cs_psum[:])
        emit_tap("d_cs", pack2[:, 0:F], [C, F])
        nc.vector.tensor_copy(pack2[:, F:F + 1], cnt_sb[:])
        b2_in = dram.tile([C, F + 1], f32)
        b2_out = dram.tile([NCORES, C, F + 1], f32, addr_space="Shared")
        nc.scalar.dma_start(b2_in[:], pack2[:])
        nc.gpsimd.collective_compute(
            "AllGather", Alu.bypass,
            replica_groups=[list(range(NCORES))],
            ins=[b2_in[:].opt()], outs=[b2_out[:].opt()])

        # ============ global merge ============
        gcand = sb.tile([B, NCORES * 8 * R2], f32)
        nc.scalar.dma_start(
            gcand[:].rearrange("b (ci r) -> b ci r", ci=NCORES),
            b1_out[:].rearrange("ci b r -> b ci r"))
        emit_tap("d_gcand", gcand[:], [B, NCORES * 8 * R2])
        G = sb.tile([B, KE], f32)
        for r in range(R3):
            sl = G[:, r * 8:(r + 1) * 8]
            nc.vector.max(out=sl, in_=gcand[:])
            nc.vector.match_replace(out=gcand[:], in_to_replace=sl,
                                    in_values=gcand[:], imm_value=SENT)
        emit_tap("d_G", G[:], [B, KE])

        # ============ index recovery (batched over b) ============
        bG_ps = ps.tile([128, KE], f32, tag="psa")
        nc.tensor.matmul(bG_ps[:], lhsT=L4[:], rhs=G[:], start=True, stop=True)
        bG = sb.tile([128, KE], f32)
        nc.vector.tensor_copy(bG[:], bG_ps[:])

        idxu = sb.tile([128, KE], u32)
        for r in range(R3):
            nc.vector.max_index(out=idxu[:, r * 8:(r + 1) * 8],
                                in_max=bG[:, r * 8:(r + 1) * 8],
                                in_values=Worig[:])
        idxf = sb.tile([128, KE], f32)
        nc.vector.tensor_copy(idxf[:], idxu[:])
        nc.vector.tensor_tensor(idxf[:], idxf[:],
                                p1024[:].to_broadcast([128, KE]), op=Alu.add)
        nc.vector.tensor_tensor(idxf[:], idxf[:],
                                bsub[:].to_broadcast([128, KE]),
                                op=Alu.subtract)
        tidx_ps = ps.tile([KE, 128], f32, tag="psa")
        nc.tensor.transpose(tidx_ps[:], idxf[:], ident[:])
        tidx = sb.tile([KE, 128], f32)
        nc.vector.tensor_copy(tidx[:], tidx_ps[:])
        lminT = sb.tile([KE, B], f32)
        for b in range(B):
            nc.vector.tensor_reduce(lminT[:, b:b + 1],
                                    tidx[:, 32 * b:32 * b + 32],
                                    axis=Axis.X, op=Alu.min)
        emit_tap("d_lminT", lminT[:], [KE, B])

        def floor_inplace(x_f, tg):
            xi = sb2.tile([KE, B], i32, tag=tg + "i")
            nc.vector.tensor_copy(xi[:], x_f[:])
            xr = sb2.tile([KE, B], f32, tag=tg + "r")
            nc.vector.tensor_copy(xr[:], xi[:])
            corr = sb2.tile([KE, B], f32, tag=tg + "c")
            nc.vector.tensor_tensor(corr[:], xr[:], x_f[:], op=Alu.is_gt)
            nc.vector.tensor_tensor(x_f[:], xr[:], corr[:], op=Alu.subtract)

        # s_dram index: b*VL + v ; embT index: b*VL + 8192h + 64p + t
        idxs_f = sb.tile([KE, B], f32)
        nc.vector.tensor_tensor(idxs_f[:], lminT[:], bcol[:], op=Alu.add)
        nc.vector.tensor_scalar_min(idxs_f[:], idxs_f[:], NOTFOUND_CLAMP)
        idxs_all = sb.tile([KE, B], i32)
        nc.vector.tensor_copy(idxs_all[:], idxs_f[:])
        h_f = sb.tile([KE, B], f32)
        nc.vector.tensor_scalar(h_f[:], lminT[:], 1.0 / 8192.0, None,
                                op0=Alu.mult)
        floor_inplace(h_f, "fh")
        r_f = sb.tile([KE, B], f32)
        nc.vector.scalar_tensor_tensor(r_f[:], in0=h_f[:], scalar=-8192.0,
                                       in1=lminT[:], op0=Alu.mult,
                                       op1=Alu.add)
        t_f = sb.tile([KE, B], f32)
        nc.vector.tensor_scalar(t_f[:], r_f[:], 1.0 / 128.0, None,
                                op0=Alu.mult)
        floor_inplace(t_f, "ft")
        p_f = sb.tile([KE, B], f32)
        nc.vector.scalar_tensor_tensor(p_f[:], in0=t_f[:], scalar=-128.0,
                                       in1=r_f[:], op0=Alu.mult, op1=Alu.add)
        row_f = sb.tile([KE, B], f32)
        nc.vector.scalar_tensor_tensor(row_f[:], in0=p_f[:], scalar=64.0,
                                       in1=t_f[:], op0=Alu.mult, op1=Alu.add)
        nc.vector.scalar_tensor_tensor(row_f[:], in0=h_f[:], scalar=8192.0,
                                       in1=row_f[:], op0=Alu.mult,
                                       op1=Alu.add)
        nc.vector.tensor_tensor(row_f[:], row_f[:], bcol[:], op=Alu.add)
        nc.vector.tensor_scalar_min(row_f[:], row_f[:], NOTFOUND_CLAMP)
        idxe_all = sb.tile([KE, B], i32)
        nc.vector.tensor_copy(idxe_all[:], row_f[:])
        own_all = sb.tile([KE, B], f32)
        nc.vector.tensor_scalar(own_all[:], lminT[:], 1.0e6, None,
                                op0=Alu.is_le)

        # ============ gathers ============

        he_all = sb.tile([KE, B, F], fp8)
        nc.vector.memset(he_all[:], 0.0)
        sel_all = sb.tile([KE, B], f32)
        nc.vector.memset(sel_all[:], 0.0)
        for b in range(B):
            nc.gpsimd.indirect_dma_start(
                out=he_all[:, b, :], out_offset=None,
                in_=embT_dram[:],
                in_offset=IndirectOffsetOnAxis(ap=idxe_all[:, b:b + 1],
                                               axis=0),
                bounds_check=B * VL - 1, oob_is_err=False)
            nc.gpsimd.indirect_dma_start(
                out=sel_all[:, b:b + 1], out_offset=None,
                in_=s_dram[:],
                in_offset=IndirectOffsetOnAxis(ap=idxs_all[:, b:b + 1],
                                               axis=0),
                bounds_check=B * VL - 1, oob_is_err=False)
        emit_tap("d_he0", he_all[:, 0, :], [KE, F], fp8)
        emit_tap("d_sel0", sel_all[:, 0:1], [KE, 1])

        # ============ EMA -> nar ============
        g2 = sb.tile([NCORES, C * (F + 1)], f32)
        nc.scalar.dma_start(g2[:], b2_out[:])
        gsum_ps = ps.tile([1, C * (F + 1)], f32, tag="psa")
        nc.tensor.matmul(gsum_ps[:], lhsT=ones8[:], rhs=g2[:],
                         start=True, stop=True)
        gsum = sb.tile([1, C * (F + 1)], f32)
        nc.vector.tensor_copy(gsum[:], gsum_ps[:])
        emit_tap("d_gsum", gsum[:], [1, C * (F + 1)])
        cs_g = gsum[:].rearrange("a (c f) -> a c f", c=C)[:, :, 0:F]
        cnt_g = gsum[:].rearrange("a (c f) -> a c f", c=C)[:, :, F:F + 1]

        cntc = sb.tile([1, C, 1], f32)
        nc.vector.tensor_scalar_max(cntc[:], cnt_g, 1.0)
        recip = sb.tile([1, C, 1], f32)
        nc.vector.reciprocal(recip[:], cntc[:])
        mean = sb.tile([1, C, F], f32)
        nc.vector.tensor_tensor(mean[:], cs_g,
                                recip[:].to_broadcast([1, C, F]), op=Alu.mult)
        avg_sb = sb.tile([1, C, F], f32)
        nc.scalar.dma_start(avg_sb[:], avg_d)
        cgt = sb.tile([1, C, 1], f32)
        nc.vector.tensor_scalar(cgt[:], cnt_g, 0.0, None, op0=Alu.is_gt)
        # avg_new = avg + cgt * theta * (mean - avg)
        t_ema = sb.tile([1, C, F], f32)
        nc.vector.tensor_tensor(t_ema[:], mean[:], avg_sb[:],
                                op=Alu.subtract)
        nc.vector.scalar_tensor_tensor(t_ema[:], in0=t_ema[:],
                                       scalar=EMA_THETA,
                                       in1=cgt[:].to_broadcast([1, C, F]),
                                       op0=Alu.mult, op1=Alu.mult)
        avg_new = sb.tile([1, C, F], f32)
        nc.vector.tensor_tensor(avg_new[:], avg_sb[:], t_ema[:], op=Alu.add)

        narb_ps = ps.tile([KE, (C - 1) * F], f32, tag="psa")
        nc.tensor.matmul(narb_ps[:], lhsT=ones104[:], rhs=avg_new[:, 1:C, :],
                         start=True, stop=True)
        narb = sb.tile([KE, C - 1, F], f32)
        nc.vector.tensor_copy(narb[:], narb_ps[:])
        emit_tap("d_narb", narb[:], [KE, (C - 1) * F])

        # ============ loss epilogue (batched over b and c) ============
        hef = sb.tile([KE, B, F], f32)
        nc.vector.tensor_copy(hef[:], he_all[:])
        prod = sb.tile([KE, B, C - 1, F], f32)
        nc.vector.tensor_tensor(
            prod[:],
            hef[:].rearrange("k b (o f) -> k b o f", o=1).to_broadcast(
                [KE, B, C - 1, F]),
            narb[:].rearrange("k (o c) f -> k o c f", o=1).to_broadcast(
                [KE, B, C - 1, F]),
            op=Alu.mult)
        ex = sb.tile([KE, B, C - 1, F], f32)
        nc.scalar.activation(ex[:], prod[:], Act.Exp, scale=1.0 / TAU)
        accl = sb.tile([KE, B, F], f32)
        nc.vector.tensor_tensor(accl[:], ex[:, :, 0, :], ex[:, :, 1, :],
                                op=Alu.add)
        nc.vector.tensor_tensor(accl[:], accl[:], ex[:, :, 2, :], op=Alu.add)
        lnv = sb.tile([KE, B, F], f32)
        nc.scalar.activation(lnv[:], accl[:], Act.Ln)
        lsum = sb.tile([KE, B], f32)
        nc.vector.reduce_sum(lsum[:], lnv[:], axis=Axis.X)
        nc.vector.tensor_scalar_mul(lsum[:], lsum[:], -1.0)
        nc.vector.tensor_tensor(lsum[:], lsum[:], sel_all[:], op=Alu.mult)
        losscols = sb.tile([KE, B], f32)
        nc.vector.tensor_tensor(losscols[:], lsum[:], own_all[:], op=Alu.mult)

        final_ps = ps.tile([1, B * KE], f32, tag="psa")
        for b in range(B):
            nc.tensor.matmul(final_ps[:, b * KE:(b + 1) * KE],
                             lhsT=losscols[:, b:b + 1], rhs=ident[:KE, :KE],
                             start=True, stop=True)
        final = sb.tile([1, B * KE], f32)
        nc.vector.tensor_copy(final[:], final_ps[:])
        nc.sync.dma_start(out_d, final[:])


_NC_CACHE = {}


def _get_graph():
    if "nc" not in _NC_CACHE:
        _NC_CACHE["nc"] = build_graph()
    return _NC_CACHE["nc"]


def kernel(proba, y, embeddings, avg_repr):
    from concourse.bass_utils import run_bass_kernel_spmd

    proba = np.asarray(proba, dtype=np.float32)
    y = np.asarray(y, dtype=np.int32)
    embeddings = np.asarray(embeddings, dtype=np.float32)
    avg_repr = np.asarray(avg_repr, dtype=np.float32)

    nc = _get_graph()
    in_maps = []
    for ci in range(NCORES):
        sl = slice(ci * VL, (ci + 1) * VL)
        in_maps.append({
            "proba": np.ascontiguousarray(proba[:, :, sl]),
            "y": np.ascontiguousarray(y[:, :, sl]),
            "embeddings": np.ascontiguousarray(embeddings[:, :, sl]),
            "avg_repr": avg_repr,
        })
    res = run_bass_kernel_spmd(nc, in_maps, core_ids=list(range(NCORES)))
    parts = [res.results[ci]["out"] for ci in range(NCORES)]
    total = np.sum(parts, axis=0).astype(np.float32)
    return total[:, :K]
